# revision 21
# baseline (speedup 1.0000x reference)
"""Contrastive loss on 8 TRN2 cores — v6 (fat-row A2A, lean front).

v5 -> v6: collective payloads redeclared [128, 8w] (byte-identical flat
layout, 8x fewer+fatter rows -> A2A drops to ~5us); x DMAs issued first
so tiles arrive at full HBM pace; h-split dropped (it caused ACT
table-set ping-pong: Square binds to exp_and_others, Abs_reciprocal_sqrt
to its own set); per-chunk post work interleaved into the ship loop so
exps ride the A2A chain instead of stacking at the tail.
"""

import numpy as np
import ml_dtypes

import concourse.bacc as bacc
import concourse.mybir as mybir
import concourse.tile as tile
from concourse import bass_utils

F32 = mybir.dt.float32
F8E5 = mybir.dt.float8e5
BF16 = mybir.dt.bfloat16
FP8 = mybir.dt.float8e4
AF = mybir.ActivationFunctionType
ALU = mybir.AluOpType
PM = mybir.MatmulPerfMode

B = 1024
R = 2 * B
NCORES = 8
KT = 8
S = 16.0
INV_T_S2 = 2.0 / (S * S)
CH1 = 1024
CH2 = 1152

CHUNKS = [("A", 0, 512), ("A", 512, 1024),
          ("B", 0, 512), ("B", 512, 1024), ("B", 1024, 1152)]

_CACHE = {}


def _pieces(c, lo, hi):
    """Segment pieces of cc2-local cols [lo,hi) for pair c."""
    wa2 = 1024 - 128 * c
    tb = 15 - c
    segs = []
    a0, a1 = max(lo, 0), min(hi, wa2)
    if a1 > a0:
        segs.append((a0 - lo, 128 * c + CH1 + a0, a1 - a0, c))
    b0, b1 = max(lo, wa2), min(hi, CH2)
    if b1 > b0:
        segs.append((b0 - lo, 128 * tb + (b0 - wa2), b1 - b0, tb))
    out = []
    for po, sc, w, tr in segs:
        off = 0
        while off < w:
            ww = min(512, w - off)
            out.append((po + off, sc + off, ww, tr))
            off += ww
    return out


def _chunk_units(space, lo, hi):
    units = []
    for u in range(8):
        if space == "A":
            units.append([(0, 128 * u + lo, hi - lo, u)])
        else:
            units.append(_pieces(u, lo, hi))
    return units


def _build_nc():
    if "nc" in _CACHE:
        return _CACHE["nc"]
    nc = bacc.Bacc("TRN2", target_bir_lowering=False, debug=False,
                   num_devices=NCORES)

    x = nc.dram_tensor("x", [KT, 128, 2 * R], FP8, kind="ExternalInput")
    sel = nc.dram_tensor("sel", [128, 256], FP8, kind="ExternalInput")
    eye = nc.dram_tensor("eye", [128, 128], BF16, kind="ExternalInput")
    msk = nc.dram_tensor("msk", [2, 128, CH2], BF16, kind="ExternalInput")
    rt = nc.dram_tensor("rt", [16, 16], BF16, kind="ExternalInput")
    cm = nc.dram_tensor("cm", [2, 128, 16], F32, kind="ExternalInput")
    msel = nc.dram_tensor("msel", [128, 1], F32, kind="ExternalInput")
    y = nc.dram_tensor("y", [1, 1], F32, kind="ExternalOutput")

    # fat-row collective payloads: [128, 8w]; shard u = rows [16u,16u+16)
    # holds the same flat bytes as rows [128u,128(u+1)) of a [1024, w]
    # layout, so unit-major views below stay byte-identical.
    cc = []
    for ci, (space, lo, hi) in enumerate(CHUNKS):
        w = hi - lo
        cin = nc.dram_tensor(f"cc{ci}_in", [128, 8 * w], F8E5)
        cout = nc.dram_tensor(f"cc{ci}_out", [128, 8 * w], F8E5)
        cc.append((cin, cout))
    ccd_in = nc.dram_tensor("ccd_in", [16, 128], F32)
    ccd_out = nc.dram_tensor("ccd_out", [16, 128], F32)
    wu_in = nc.dram_tensor("wu_in", [16, 16], F8E5)
    wu_out = nc.dram_tensor("wu_out", [16, 16], F8E5)
    grp = [list(range(NCORES))]

    with tile.TileContext(nc) as tc:
        with tc.tile_pool(name="x8", bufs=KT) as px8, \
             tc.tile_pool(name="sq", bufs=4) as psq, \
             tc.tile_pool(name="pers", bufs=1) as pers, \
             tc.tile_pool(name="simsb", bufs=2) as psim, \
             tc.tile_pool(name="acc", bufs=2) as pacc, \
             tc.tile_pool(name="sum", bufs=2) as psum_pool, \
             tc.tile_pool(name="loss", bufs=1) as plo, \
             tc.tile_pool(name="ps", bufs=7, space="PSUM") as pps:

            # ---- all x DMAs first (clean queue round-robin) ----
            xb = []
            for k in range(KT):
                t = px8.tile([128, 2 * R], FP8, tag="x8")
                nc.sync.dma_start(t[0:64, :], x[k, 0:64, :])
                nc.sync.dma_start(t[64:128, :], x[k, 64:128, :])
                xb.append(t)

            selb = pers.tile([128, 256], FP8, tag="selb")
            nc.sync.dma_start(selb[:], sel[:])
            eyeb = pers.tile([128, 128], BF16, tag="eyeb")
            nc.sync.dma_start(eyeb[:], eye[:])
            mskb = pers.tile([128, 2 * CH2], BF16, tag="mskb")
            nc.sync.dma_start(mskb[:, 0:CH2], msk[0, :, :])
            nc.sync.dma_start(mskb[:, CH2:2 * CH2], msk[1, :, :])
            rtb = pers.tile([16, 16], BF16, tag="rtb")
            nc.sync.dma_start(rtb[:], rt[:])
            cmb = pers.tile([128, 32], F32, tag="cmb")
            nc.sync.dma_start(cmb[:, 0:16], cm[0, :, :])
            nc.sync.dma_start(cmb[:, 16:32], cm[1, :, :])
            mselb = pers.tile([128, 1], F32, tag="mselb")
            nc.sync.dma_start(mselb[:], msel[:])

            # ACT table warm: Square's set (exp_and_others)
            junk = pers.tile([128, 16], F32, tag="junk")
            nc.vector.memset(junk[:], 1.0)
            junko = pers.tile([128, 16], F32, tag="junko")
            nc.scalar.activation(junko[:], junk[:], AF.Square)

            ones1 = pers.tile([128, 1], BF16, tag="ones1")
            nc.vector.memset(ones1[:], 1.0)
            negf2 = pers.tile([128, 1], F32, tag="negf2")
            nc.vector.memset(negf2[:], -2.0 * INV_T_S2)

            wub = pers.tile([16, 16], F8E5, tag="wub")
            nc.vector.memset(wub[:], 1.0)
            nc.gpsimd.dma_start(wu_in[:], wub[:])
            nc.gpsimd.collective_compute(
                "AllToAll", ALU.bypass, replica_groups=grp,
                ins=[wu_in[:].opt()], outs=[wu_out[:].opt()])

            selv = selb[:].rearrange("p (two j) -> p two j", two=2)
            scale_t = pers.tile([128, R], FP8, tag="scale_t")

            def vk(k):
                return xb[k][:].rearrange("p (two r) -> p two r", two=2)

            # ---- squares (ACT even / DVE odd) + ssq matmuls ----
            ssq = [pps.tile([128, 512], F32, tag="ps", name=f"ssq{j}")
                   for j in range(4)]
            for k in range(KT):
                sq = psq.tile([128, 2 * R], FP8, tag="sq")
                src = xb[k][:]
                if k % 2 == 0:
                    nc.scalar.activation(sq[:], src, AF.Square)
                else:
                    nc.vector.tensor_tensor(sq[:], src, src, ALU.mult)
                sqv = sq[:].rearrange("p (two r) -> p two r", two=2)
                for j in range(4):
                    nc.tensor.matmul(ssq[j][:], selv,
                                     sqv[:, :, 512 * j:512 * (j + 1)],
                                     start=(k == 0), stop=(k == KT - 1),
                                     perf_mode=PM.DoubleRow)
            for j in range(4):
                nc.scalar.activation(scale_t[:, 512 * j:512 * (j + 1)],
                                     ssq[j][:], AF.Abs_reciprocal_sqrt,
                                     scale=128.0 / (S * S))
            # pull the exp/copy table set back in off the critical path
            nc.scalar.activation(junko[:], junk[:], AF.Exp)

            # gram machinery ------------------------------------------
            chunk_tiles = {}

            def gram_part(ci, uis, ks, unit_major=True):
                space, lo, hi = CHUNKS[ci]
                w = hi - lo
                units = _chunk_units(space, lo, hi)
                if ci not in chunk_tiles:
                    chunk_tiles[ci] = [None] * 8
                for ui in uis:
                    if chunk_tiles[ci][ui] is None:
                        chunk_tiles[ci][ui] = pps.tile(
                            [128, w], F32, tag="ps", name=f"g{ci}_{ui}")
                order = ([(ui, k) for ui in uis for k in ks] if unit_major
                         else [(ui, k) for k in ks for ui in uis])
                for ui, k in order:
                    v = vk(k)
                    pt = chunk_tiles[ci][ui]
                    for po, sc, ww, tr in units[ui]:
                        lhsT = v[:, :, 128 * tr:128 * (tr + 1)]
                        nc.tensor.matmul(pt[:, po:po + ww], lhsT,
                                         v[:, :, sc:sc + ww],
                                         start=(k == 0),
                                         stop=(k == KT - 1),
                                         perf_mode=PM.DoubleRow)

            def ship_chunk(ci):
                space, lo, hi = CHUNKS[ci]
                w = hi - lo
                cin, cout = cc[ci]
                wide = psim.tile([128, 8 * w], F8E5, tag="simsb",
                                 name=f"wide{ci}")
                for ui in range(8):
                    nc.scalar.activation(wide[:, ui * w:(ui + 1) * w],
                                         chunk_tiles[ci][ui][:], AF.Copy)
                # unit-major [p, u, w] view of the fat-row [128, 8w] dram:
                # offset (u*128+p)*w, byte-identical to a [1024, w] layout
                cv = cin[:].rearrange("(u a2) (b w) -> (a2 b) u w",
                                      u=8, b=8)
                wv = wide[:].rearrange("p (u w) -> p u w", u=8)
                nc.gpsimd.dma_start(cv[:, 0:4, :], wv[:, 0:4, :])
                nc.gpsimd.dma_start(cv[:, 4:8, :], wv[:, 4:8, :])
                nc.gpsimd.collective_compute(
                    "AllToAll", ALU.bypass, replica_groups=grp,
                    ins=[cin[:].opt()], outs=[cout[:].opt()])

            # norm + chunk0 interleaved per k; then chunks 1-4
            for k in range(KT):
                for s in range(2):
                    sl = xb[k][:, s * R:(s + 1) * R]
                    nc.vector.tensor_tensor(sl, sl, scale_t[:], ALU.mult)
                gram_part(0, list(range(8)), [k], unit_major=False)
            ship_chunk(0)
            for ci in range(1, len(CHUNKS)):
                gram_part(ci, list(range(4)), list(range(KT)))
                gram_part(ci, list(range(4, 8)), list(range(KT)))
                ship_chunk(ci)

            # ---- per-chunk post-collective loss work ----
            rs_parts = []
            rsA2_parts = []
            expdB_parts = []
            holders = {}
            pc_idx = 0
            pc_sb = plo.tile([128, 16], BF16, tag="pc_sb")

            def post_chunk(ci):
                nonlocal pc_idx
                space, lo, hi = CHUNKS[ci]
                w = hi - lo
                cin, cout = cc[ci]
                ld = psum_pool.tile([128, 8 * w], F8E5, tag="ld",
                                    name=f"ld{ci}", bufs=2)
                lv = ld[:].rearrange("p (s w) -> p s w", s=8)
                ov = cout[:].rearrange("(s a2) (b w) -> (a2 b) s w",
                                       s=8, b=8)
                nc.gpsimd.dma_start(lv[:, 0:4, :], ov[:, 0:4, :])
                nc.gpsimd.dma_start(lv[:, 4:8, :], ov[:, 4:8, :])
                t4 = []
                for a in range(4):
                    tt = psum_pool.tile([128, w], BF16, tag="t4",
                                        name=f"t4_{ci}_{a}", bufs=4)
                    nc.vector.tensor_tensor(tt[:], lv[:, 2 * a, :],
                                            lv[:, 2 * a + 1, :], ALU.add)
                    t4.append(tt)
                t2 = []
                for a in range(2):
                    tt = psum_pool.tile([128, w], BF16, tag="t2",
                                        name=f"t2_{ci}_{a}", bufs=2)
                    nc.vector.tensor_tensor(tt[:], t4[2 * a][:],
                                            t4[2 * a + 1][:], ALU.add)
                    t2.append(tt)
                sim = psum_pool.tile([128, w], BF16, tag="sim",
                                     name=f"sim{ci}", bufs=2)
                nc.vector.tensor_tensor(sim[:], t2[0][:], t2[1][:], ALU.add)

                if space == "B" and lo == 0:
                    scrP = pacc.tile([128, 128], BF16, tag="scrP", bufs=1)
                    possum = plo.tile([128, 1], F32, tag="possum")
                    nc.vector.scalar_tensor_tensor(
                        scrP[:], sim[:, 0:128], 1.0, eyeb[:],
                        ALU.mult, ALU.mult, accum_out=possum[:])
                    holders["possum"] = possum

                ex = psum_pool.tile([128, w], BF16, tag="ex",
                                    name=f"ex{ci}", bufs=5)
                rs = plo.tile([128, 1], F32, tag=f"rs{ci}")
                nc.scalar.activation(ex[:], sim[:], AF.Exp, scale=INV_T_S2,
                                     accum_out=rs[:])
                rs_parts.append((space, rs))

                if space == "A" and lo == 0:
                    scrA = pacc.tile([128, 128], BF16, tag="scrA", bufs=1)
                    expdA = plo.tile([128, 1], F32, tag="expdA")
                    nc.vector.scalar_tensor_tensor(
                        scrA[:], ex[:, 0:128], 1.0, eyeb[:],
                        ALU.mult, ALU.mult, accum_out=expdA[:])
                    holders["expdA"] = expdA
                if space == "B":
                    scr0 = pacc.tile([128, w], BF16, tag="scr0",
                                     name=f"scr0_{ci}", bufs=2)
                    ra = plo.tile([128, 1], F32, tag=f"ra{ci}")
                    nc.vector.scalar_tensor_tensor(
                        scr0[:], ex[:], 1.0, mskb[:, lo:hi],
                        ALU.mult, ALU.mult, accum_out=ra[:])
                    rsA2_parts.append(ra)
                    scr1 = pacc.tile([128, w], BF16, tag="scr1",
                                     name=f"scr1_{ci}", bufs=2)
                    rb = plo.tile([128, 1], F32, tag=f"rb{ci}")
                    nc.vector.scalar_tensor_tensor(
                        scr1[:], ex[:], 1.0, mskb[:, CH2 + lo:CH2 + hi],
                        ALU.mult, ALU.mult, accum_out=rb[:])
                    expdB_parts.append(rb)

                blocks = list(range(w // 128))
                if space == "A" and lo == 0:
                    blocks = blocks[1:]
                nb = len(blocks)
                ps4 = pps.tile([128, nb], F32, tag="pc", name=f"pc{ci}",
                               bufs=1)
                for bi, j in enumerate(blocks):
                    nc.tensor.matmul(ps4[:, bi:bi + 1],
                                     ex[:, 128 * j:128 * (j + 1)],
                                     ones1[:], start=True, stop=True)
                nc.scalar.activation(pc_sb[:, pc_idx:pc_idx + nb],
                                     ps4[:], AF.Copy)
                pc_idx += nb

            for ci in range(len(CHUNKS)):
                post_chunk(ci)

            possum = holders["possum"]
            expdA = holders["expdA"]

            # colsum redistribution via one end transpose + rt matmul
            ps_t = pps.tile([128, 128], BF16, tag="ps", name="ps_t")
            nc.tensor.transpose(ps_t[0:16, :], pc_sb[:], eyeb[:])
            pt_sb = plo.tile([16, 128], BF16, tag="pt_sb")
            nc.vector.tensor_copy(pt_sb[:], ps_t[0:16, :])
            ps_add = pps.tile([128, 16], F32, tag="ps", name="ps_add")
            nc.tensor.matmul(ps_add[:], pt_sb[:], rtb[:], start=True,
                             stop=True)

            # ---- combine denominators ----
            denA = plo.tile([128, 1], F32, tag="denA")
            denB = plo.tile([128, 1], F32, tag="denB")
            rsA1 = plo.tile([128, 1], F32, tag="rsA1")
            a_parts = [r for sp, r in rs_parts if sp == "A"]
            b_parts = [r for sp, r in rs_parts if sp == "B"]
            nc.vector.tensor_tensor(rsA1[:], a_parts[0][:], a_parts[1][:],
                                    ALU.add)
            rs2t = plo.tile([128, 1], F32, tag="rs2t")
            nc.vector.tensor_tensor(rs2t[:], b_parts[0][:], b_parts[1][:],
                                    ALU.add)
            nc.vector.tensor_tensor(rs2t[:], rs2t[:], b_parts[2][:], ALU.add)
            rsA2 = plo.tile([128, 1], F32, tag="rsA2")
            nc.vector.tensor_tensor(rsA2[:], rsA2_parts[0][:],
                                    rsA2_parts[1][:], ALU.add)
            nc.vector.tensor_tensor(rsA2[:], rsA2[:], rsA2_parts[2][:],
                                    ALU.add)
            expdB = plo.tile([128, 1], F32, tag="expdB")
            nc.vector.tensor_tensor(expdB[:], expdB_parts[0][:],
                                    expdB_parts[1][:], ALU.add)
            nc.vector.tensor_tensor(expdB[:], expdB[:], expdB_parts[2][:],
                                    ALU.add)
            nc.vector.tensor_tensor(denA[:], rsA1[:], rsA2[:], ALU.add)
            nc.vector.tensor_sub(denA[:], denA[:], expdA[:])
            nc.vector.tensor_sub(denB[:], rs2t[:], rsA2[:])
            nc.vector.tensor_sub(denB[:], denB[:], expdB[:])

            den16 = plo.tile([128, 16], F32, tag="den16")
            nc.vector.scalar_tensor_tensor(
                den16[:], cmb[:, 0:16], 1.0,
                denA[:].to_broadcast((128, 16)), ALU.mult, ALU.mult)
            t2m = plo.tile([128, 16], F32, tag="t2m")
            nc.vector.scalar_tensor_tensor(
                t2m[:], cmb[:, 16:32], 1.0,
                denB[:].to_broadcast((128, 16)), ALU.mult, ALU.mult)
            nc.vector.tensor_tensor(den16[:], den16[:], t2m[:], ALU.add)
            nc.vector.tensor_tensor(den16[:], den16[:], ps_add[:], ALU.add)

            civ = ccd_in[:].rearrange("a (b j) -> (a b) j", b=8)
            nc.gpsimd.dma_start(civ, den16[:])
            nc.gpsimd.collective_compute(
                "AllReduce", ALU.add, replica_groups=grp,
                ins=[ccd_in[:].opt()], outs=[ccd_out[:].opt()])

            # hide the natural_log table load under the AllReduce
            junk3 = pers.tile([128, 16], F32, tag="junk3")
            nc.scalar.activation(junk3[:], junk[:], AF.Ln)

            denf = plo.tile([128, 16], F32, tag="denf")
            cov = ccd_out[:].rearrange("a (b j) -> (a b) j", b=8)
            nc.gpsimd.dma_start(denf[:], cov)
            lnj = plo.tile([128, 16], F32, tag="lnj")
            lnacc = plo.tile([128, 1], F32, tag="lnacc")
            nc.scalar.activation(lnj[:], denf[:], AF.Ln, accum_out=lnacc[:])

            loss_ps = pps.tile([1, 1], F32, tag="ps", name="loss_ps")
            nc.tensor.matmul(loss_ps[:], lnacc[:], mselb[:],
                             start=True, stop=False)
            nc.tensor.matmul(loss_ps[:], possum[:], negf2[:],
                             start=False, stop=True)
            out_sb = pers.tile([1, 1], F32, tag="outsb")
            nc.vector.tensor_copy(out_sb[:], loss_ps[:])
            nc.sync.dma_start(y[:], out_sb[:])

    nc.compile()
    _CACHE["nc"] = nc
    return nc


def _make_inputs(emb_i, emb_j):
    e = np.concatenate([np.asarray(emb_i, np.float32),
                        np.asarray(emb_j, np.float32)], axis=0)
    sel = np.zeros((128, 2, 128), np.float32)
    for p in range(128):
        sel[p, :, np.arange(p % 16, 128, 16)] = 1.0
    sel = sel.reshape(128, 256).astype(ml_dtypes.float8_e4m3)
    eye = np.eye(128, dtype=np.float32).astype(ml_dtypes.bfloat16)

    in_maps = []
    for c in range(NCORES):
        loc = e[:, :, 16 * c:16 * (c + 1)]
        t = loc.reshape(R, 8, 8, 2, 16)
        t = t.transpose(1, 2, 4, 3, 0)
        x = np.ascontiguousarray(t).reshape(KT, 128, 2 * R).astype(
            ml_dtypes.float8_e4m3)

        wa2 = 1024 - 128 * c
        msk = np.zeros((2, 128, CH2), np.float32)
        msk[0, :, 0:wa2] = 1.0
        jd = 8 - c
        msk[1, np.arange(128), 128 * jd + np.arange(128)] = 1.0

        rt_m = np.zeros((16, 16), np.float32)
        for col in range(16):
            if col < 15 - c:
                rt_m[col, c + 1 + col] = 1.0
            elif col >= 16 - c:
                rt_m[col, col] = 1.0
        cm_m = np.zeros((2, 128, 16), np.float32)
        cm_m[0, :, c] = 1.0
        cm_m[1, :, 15 - c] = 1.0

        msel_m = np.zeros((128, 1), np.float32)
        msel_m[16 * c:16 * (c + 1), 0] = 1.0

        in_maps.append({
            "x": x, "sel": sel, "eye": eye,
            "msk": msk.astype(ml_dtypes.bfloat16),
            "rt": rt_m.astype(ml_dtypes.bfloat16),
            "cm": cm_m.astype(np.float32),
            "msel": msel_m,
        })
    return in_maps


def run(emb_i, emb_j, **spmd_kwargs):
    nc = _build_nc()
    in_maps = _make_inputs(emb_i, emb_j)
    res = bass_utils.run_bass_kernel_spmd(
        nc, in_maps, core_ids=list(range(NCORES)), **spmd_kwargs)
    total = sum(float(r["y"][0, 0]) for r in res.results)
    return np.array(total / R, dtype=np.float32), res


def kernel(emb_i, emb_j):
    loss, _ = run(emb_i, emb_j)
    return loss


# revision 24
# speedup vs baseline: 1.0149x; 1.0149x over previous
"""Contrastive loss on 8 TRN2 cores — v6 (fat-row A2A, lean front).

v5 -> v6: collective payloads redeclared [128, 8w] (byte-identical flat
layout, 8x fewer+fatter rows -> A2A drops to ~5us); x DMAs issued first
so tiles arrive at full HBM pace; h-split dropped (it caused ACT
table-set ping-pong: Square binds to exp_and_others, Abs_reciprocal_sqrt
to its own set); per-chunk post work interleaved into the ship loop so
exps ride the A2A chain instead of stacking at the tail.
"""

import numpy as np
import ml_dtypes

import concourse.bacc as bacc
import concourse.mybir as mybir
import concourse.tile as tile
from concourse import bass_utils

F32 = mybir.dt.float32
F8E5 = mybir.dt.float8e5
BF16 = mybir.dt.bfloat16
FP8 = mybir.dt.float8e4
AF = mybir.ActivationFunctionType
ALU = mybir.AluOpType
PM = mybir.MatmulPerfMode

B = 1024
R = 2 * B
NCORES = 8
KT = 8
S = 16.0
INV_T_S2 = 2.0 / (S * S)
CH1 = 1024
CH2 = 1152

CHUNKS = [("A", 0, 512), ("A", 512, 1024),
          ("B", 0, 512), ("B", 512, 1024), ("B", 1024, 1152)]

_CACHE = {}


def _pieces(c, lo, hi):
    """Segment pieces of cc2-local cols [lo,hi) for pair c."""
    wa2 = 1024 - 128 * c
    tb = 15 - c
    segs = []
    a0, a1 = max(lo, 0), min(hi, wa2)
    if a1 > a0:
        segs.append((a0 - lo, 128 * c + CH1 + a0, a1 - a0, c))
    b0, b1 = max(lo, wa2), min(hi, CH2)
    if b1 > b0:
        segs.append((b0 - lo, 128 * tb + (b0 - wa2), b1 - b0, tb))
    out = []
    for po, sc, w, tr in segs:
        off = 0
        while off < w:
            ww = min(512, w - off)
            out.append((po + off, sc + off, ww, tr))
            off += ww
    return out


def _chunk_units(space, lo, hi):
    units = []
    for u in range(8):
        if space == "A":
            units.append([(0, 128 * u + lo, hi - lo, u)])
        else:
            units.append(_pieces(u, lo, hi))
    return units


def _build_nc():
    if "nc" in _CACHE:
        return _CACHE["nc"]
    nc = bacc.Bacc("TRN2", target_bir_lowering=False, debug=False,
                   num_devices=NCORES)

    x = nc.dram_tensor("x", [KT, 128, 2 * R], FP8, kind="ExternalInput")
    sel = nc.dram_tensor("sel", [128, 256], FP8, kind="ExternalInput")
    eye = nc.dram_tensor("eye", [128, 128], BF16, kind="ExternalInput")
    msk = nc.dram_tensor("msk", [2, 128, CH2], BF16, kind="ExternalInput")
    rt = nc.dram_tensor("rt", [16, 16], BF16, kind="ExternalInput")
    cm = nc.dram_tensor("cm", [2, 128, 16], F32, kind="ExternalInput")
    msel = nc.dram_tensor("msel", [128, 1], F32, kind="ExternalInput")
    y = nc.dram_tensor("y", [1, 1], F32, kind="ExternalOutput")

    # fat-row collective payloads: [128, 8w]; shard u = rows [16u,16u+16)
    # holds the same flat bytes as rows [128u,128(u+1)) of a [1024, w]
    # layout, so unit-major views below stay byte-identical.
    cc = []
    for ci, (space, lo, hi) in enumerate(CHUNKS):
        w = hi - lo
        cin = nc.dram_tensor(f"cc{ci}_in", [128, 8 * w], F8E5)
        cout = nc.dram_tensor(f"cc{ci}_out", [128, 8 * w], F8E5)
        cc.append((cin, cout))
    ccd_in = nc.dram_tensor("ccd_in", [16, 128], F32)
    ccd_out = nc.dram_tensor("ccd_out", [16, 128], F32)
    wu_in = nc.dram_tensor("wu_in", [16, 16], F8E5)
    wu_out = nc.dram_tensor("wu_out", [16, 16], F8E5)
    grp = [list(range(NCORES))]

    with tile.TileContext(nc) as tc:
        with tc.tile_pool(name="x8", bufs=KT) as px8, \
             tc.tile_pool(name="sq", bufs=4) as psq, \
             tc.tile_pool(name="pers", bufs=1) as pers, \
             tc.tile_pool(name="simsb", bufs=2) as psim, \
             tc.tile_pool(name="acc", bufs=2) as pacc, \
             tc.tile_pool(name="sum", bufs=2) as psum_pool, \
             tc.tile_pool(name="loss", bufs=1) as plo, \
             tc.tile_pool(name="ps", bufs=7, space="PSUM") as pps:

            # ---- sel first (unblocks ssq), then x split across the two
            # HWDGE queues (SP + Activation) for 2x DMA service ----
            selb = pers.tile([128, 256], FP8, tag="selb")
            nc.scalar.dma_start(selb[:], sel[:])
            xb = []
            for k in range(KT):
                t = px8.tile([128, 2 * R], FP8, tag="x8")
                eng = nc.sync if k % 2 == 0 else nc.scalar
                eng.dma_start(t[0:64, :], x[k, 0:64, :])
                eng.dma_start(t[64:128, :], x[k, 64:128, :])
                xb.append(t)
            eyeb = pers.tile([128, 128], BF16, tag="eyeb")
            nc.sync.dma_start(eyeb[:], eye[:])
            mskb = pers.tile([128, 2 * CH2], BF16, tag="mskb")
            nc.sync.dma_start(mskb[:, 0:CH2], msk[0, :, :])
            nc.sync.dma_start(mskb[:, CH2:2 * CH2], msk[1, :, :])
            rtb = pers.tile([16, 16], BF16, tag="rtb")
            nc.sync.dma_start(rtb[:], rt[:])
            cmb = pers.tile([128, 32], F32, tag="cmb")
            nc.sync.dma_start(cmb[:, 0:16], cm[0, :, :])
            nc.sync.dma_start(cmb[:, 16:32], cm[1, :, :])
            mselb = pers.tile([128, 1], F32, tag="mselb")
            nc.sync.dma_start(mselb[:], msel[:])

            # ACT table warm: Square's set (exp_and_others)
            junk = pers.tile([128, 16], F32, tag="junk")
            nc.vector.memset(junk[:], 1.0)
            junko = pers.tile([128, 16], F32, tag="junko")
            nc.scalar.activation(junko[:], junk[:], AF.Square)

            ones1 = pers.tile([128, 1], BF16, tag="ones1")
            nc.vector.memset(ones1[:], 1.0)
            negf2 = pers.tile([128, 1], F32, tag="negf2")
            nc.vector.memset(negf2[:], -2.0 * INV_T_S2)

            wub = pers.tile([16, 16], F8E5, tag="wub")
            nc.vector.memset(wub[:], 1.0)
            nc.gpsimd.dma_start(wu_in[:], wub[:])
            nc.gpsimd.collective_compute(
                "AllToAll", ALU.bypass, replica_groups=grp,
                ins=[wu_in[:].opt()], outs=[wu_out[:].opt()])

            selv = selb[:].rearrange("p (two j) -> p two j", two=2)
            scale_t = pers.tile([128, R], FP8, tag="scale_t")

            def vk(k):
                return xb[k][:].rearrange("p (two r) -> p two r", two=2)

            # ---- squares (ACT even / DVE odd) + ssq matmuls ----
            ssq = [pps.tile([128, 512], F32, tag="ps", name=f"ssq{j}")
                   for j in range(4)]
            for k in range(KT):
                sq = psq.tile([128, 2 * R], FP8, tag="sq")
                src = xb[k][:]
                if k % 2 == 0:
                    nc.scalar.activation(sq[:], src, AF.Square)
                else:
                    nc.vector.tensor_tensor(sq[:], src, src, ALU.mult)
                sqv = sq[:].rearrange("p (two r) -> p two r", two=2)
                for j in range(4):
                    nc.tensor.matmul(ssq[j][:], selv,
                                     sqv[:, :, 512 * j:512 * (j + 1)],
                                     start=(k == 0), stop=(k == KT - 1),
                                     perf_mode=PM.DoubleRow)
            for j in range(4):
                nc.scalar.activation(scale_t[:, 512 * j:512 * (j + 1)],
                                     ssq[j][:], AF.Abs_reciprocal_sqrt,
                                     scale=128.0 / (S * S))
            # pull the exp/copy table set back in off the critical path
            nc.scalar.activation(junko[:], junk[:], AF.Exp)

            # gram machinery ------------------------------------------
            chunk_tiles = {}

            def gram_part(ci, uis, ks, unit_major=True):
                space, lo, hi = CHUNKS[ci]
                w = hi - lo
                units = _chunk_units(space, lo, hi)
                if ci not in chunk_tiles:
                    chunk_tiles[ci] = [None] * 8
                for ui in uis:
                    if chunk_tiles[ci][ui] is None:
                        chunk_tiles[ci][ui] = pps.tile(
                            [128, w], F32, tag="ps", name=f"g{ci}_{ui}")
                order = ([(ui, k) for ui in uis for k in ks] if unit_major
                         else [(ui, k) for k in ks for ui in uis])
                for ui, k in order:
                    v = vk(k)
                    pt = chunk_tiles[ci][ui]
                    for po, sc, ww, tr in units[ui]:
                        lhsT = v[:, :, 128 * tr:128 * (tr + 1)]
                        nc.tensor.matmul(pt[:, po:po + ww], lhsT,
                                         v[:, :, sc:sc + ww],
                                         start=(k == 0),
                                         stop=(k == KT - 1),
                                         perf_mode=PM.DoubleRow)

            def ship_chunk(ci):
                space, lo, hi = CHUNKS[ci]
                w = hi - lo
                cin, cout = cc[ci]
                wide = psim.tile([128, 8 * w], F8E5, tag="simsb",
                                 name=f"wide{ci}")
                for ui in range(8):
                    dst = wide[:, ui * w:(ui + 1) * w]
                    if ui % 2 == 0:
                        nc.scalar.activation(dst, chunk_tiles[ci][ui][:],
                                             AF.Copy)
                    else:
                        nc.vector.tensor_copy(dst, chunk_tiles[ci][ui][:])
                # unit-major [p, u, w] view of the fat-row [128, 8w] dram:
                # offset (u*128+p)*w, byte-identical to a [1024, w] layout
                cv = cin[:].rearrange("(u a2) (b w) -> (a2 b) u w",
                                      u=8, b=8)
                wv = wide[:].rearrange("p (u w) -> p u w", u=8)
                nc.gpsimd.dma_start(cv[:, 0:4, :], wv[:, 0:4, :])
                nc.gpsimd.dma_start(cv[:, 4:8, :], wv[:, 4:8, :])
                nc.gpsimd.collective_compute(
                    "AllToAll", ALU.bypass, replica_groups=grp,
                    ins=[cin[:].opt()], outs=[cout[:].opt()])

            # norm + chunk0 interleaved per k; then chunks 1-4
            for k in range(KT):
                for s in range(2):
                    sl = xb[k][:, s * R:(s + 1) * R]
                    nc.vector.tensor_tensor(sl, sl, scale_t[:], ALU.mult)
                gram_part(0, list(range(8)), [k], unit_major=False)
            ship_chunk(0)

            # ---- per-chunk post-collective loss work ----
            rs_parts = []
            rsA2_parts = []
            expdB_parts = []
            holders = {}
            pc_idx = 0
            pc_sb = plo.tile([128, 16], BF16, tag="pc_sb")

            def post_chunk(ci):
                nonlocal pc_idx
                space, lo, hi = CHUNKS[ci]
                w = hi - lo
                cin, cout = cc[ci]
                ld = psum_pool.tile([128, 8 * w], F8E5, tag="ld",
                                    name=f"ld{ci}", bufs=2)
                lv = ld[:].rearrange("p (s w) -> p s w", s=8)
                ov = cout[:].rearrange("(s a2) (b w) -> (a2 b) s w",
                                       s=8, b=8)
                nc.gpsimd.dma_start(lv[:, 0:4, :], ov[:, 0:4, :])
                nc.gpsimd.dma_start(lv[:, 4:8, :], ov[:, 4:8, :])
                t4 = []
                for a in range(4):
                    tt = psum_pool.tile([128, w], BF16, tag="t4",
                                        name=f"t4_{ci}_{a}", bufs=4)
                    nc.vector.tensor_tensor(tt[:], lv[:, 2 * a, :],
                                            lv[:, 2 * a + 1, :], ALU.add)
                    t4.append(tt)
                t2 = []
                for a in range(2):
                    tt = psum_pool.tile([128, w], BF16, tag="t2",
                                        name=f"t2_{ci}_{a}", bufs=2)
                    nc.vector.tensor_tensor(tt[:], t4[2 * a][:],
                                            t4[2 * a + 1][:], ALU.add)
                    t2.append(tt)
                sim = psum_pool.tile([128, w], BF16, tag="sim",
                                     name=f"sim{ci}", bufs=2)
                nc.vector.tensor_tensor(sim[:], t2[0][:], t2[1][:], ALU.add)

                if space == "B" and lo == 0:
                    scrP = pacc.tile([128, 128], BF16, tag="scrP", bufs=1)
                    possum = plo.tile([128, 1], F32, tag="possum")
                    nc.vector.scalar_tensor_tensor(
                        scrP[:], sim[:, 0:128], 1.0, eyeb[:],
                        ALU.mult, ALU.mult, accum_out=possum[:])
                    holders["possum"] = possum

                ex = psum_pool.tile([128, w], BF16, tag="ex",
                                    name=f"ex{ci}", bufs=5)
                rs = plo.tile([128, 1], F32, tag=f"rs{ci}")
                nc.scalar.activation(ex[:], sim[:], AF.Exp, scale=INV_T_S2,
                                     accum_out=rs[:])
                rs_parts.append((space, rs))

                if space == "A" and lo == 0:
                    scrA = pacc.tile([128, 128], BF16, tag="scrA", bufs=1)
                    expdA = plo.tile([128, 1], F32, tag="expdA")
                    nc.vector.scalar_tensor_tensor(
                        scrA[:], ex[:, 0:128], 1.0, eyeb[:],
                        ALU.mult, ALU.mult, accum_out=expdA[:])
                    holders["expdA"] = expdA
                if space == "B":
                    scr0 = pacc.tile([128, w], BF16, tag="scr0",
                                     name=f"scr0_{ci}", bufs=2)
                    ra = plo.tile([128, 1], F32, tag=f"ra{ci}")
                    nc.vector.scalar_tensor_tensor(
                        scr0[:], ex[:], 1.0, mskb[:, lo:hi],
                        ALU.mult, ALU.mult, accum_out=ra[:])
                    rsA2_parts.append(ra)
                    scr1 = pacc.tile([128, w], BF16, tag="scr1",
                                     name=f"scr1_{ci}", bufs=2)
                    rb = plo.tile([128, 1], F32, tag=f"rb{ci}")
                    nc.vector.scalar_tensor_tensor(
                        scr1[:], ex[:], 1.0, mskb[:, CH2 + lo:CH2 + hi],
                        ALU.mult, ALU.mult, accum_out=rb[:])
                    expdB_parts.append(rb)

                blocks = list(range(w // 128))
                if space == "A" and lo == 0:
                    blocks = blocks[1:]
                nb = len(blocks)
                ps4 = pps.tile([128, nb], F32, tag="pc", name=f"pc{ci}",
                               bufs=1)
                for bi, j in enumerate(blocks):
                    nc.tensor.matmul(ps4[:, bi:bi + 1],
                                     ex[:, 128 * j:128 * (j + 1)],
                                     ones1[:], start=True, stop=True)
                nc.scalar.activation(pc_sb[:, pc_idx:pc_idx + nb],
                                     ps4[:], AF.Copy)
                pc_idx += nb

            for ci in range(1, len(CHUNKS)):
                gram_part(ci, list(range(4)), list(range(KT)))
                gram_part(ci, list(range(4, 8)), list(range(KT)))
                ship_chunk(ci)
                post_chunk(ci - 1)
            post_chunk(len(CHUNKS) - 1)

            possum = holders["possum"]
            expdA = holders["expdA"]

            # colsum redistribution via one end transpose + rt matmul
            ps_t = pps.tile([128, 128], BF16, tag="ps", name="ps_t")
            nc.tensor.transpose(ps_t[0:16, :], pc_sb[:], eyeb[:])
            pt_sb = plo.tile([16, 128], BF16, tag="pt_sb")
            nc.vector.tensor_copy(pt_sb[:], ps_t[0:16, :])
            ps_add = pps.tile([128, 16], F32, tag="ps", name="ps_add")
            nc.tensor.matmul(ps_add[:], pt_sb[:], rtb[:], start=True,
                             stop=True)

            # ---- combine denominators ----
            denA = plo.tile([128, 1], F32, tag="denA")
            denB = plo.tile([128, 1], F32, tag="denB")
            rsA1 = plo.tile([128, 1], F32, tag="rsA1")
            a_parts = [r for sp, r in rs_parts if sp == "A"]
            b_parts = [r for sp, r in rs_parts if sp == "B"]
            nc.vector.tensor_tensor(rsA1[:], a_parts[0][:], a_parts[1][:],
                                    ALU.add)
            rs2t = plo.tile([128, 1], F32, tag="rs2t")
            nc.vector.tensor_tensor(rs2t[:], b_parts[0][:], b_parts[1][:],
                                    ALU.add)
            nc.vector.tensor_tensor(rs2t[:], rs2t[:], b_parts[2][:], ALU.add)
            rsA2 = plo.tile([128, 1], F32, tag="rsA2")
            nc.vector.tensor_tensor(rsA2[:], rsA2_parts[0][:],
                                    rsA2_parts[1][:], ALU.add)
            nc.vector.tensor_tensor(rsA2[:], rsA2[:], rsA2_parts[2][:],
                                    ALU.add)
            expdB = plo.tile([128, 1], F32, tag="expdB")
            nc.vector.tensor_tensor(expdB[:], expdB_parts[0][:],
                                    expdB_parts[1][:], ALU.add)
            nc.vector.tensor_tensor(expdB[:], expdB[:], expdB_parts[2][:],
                                    ALU.add)
            nc.vector.tensor_tensor(denA[:], rsA1[:], rsA2[:], ALU.add)
            nc.vector.tensor_sub(denA[:], denA[:], expdA[:])
            nc.vector.tensor_sub(denB[:], rs2t[:], rsA2[:])
            nc.vector.tensor_sub(denB[:], denB[:], expdB[:])

            den16 = plo.tile([128, 16], F32, tag="den16")
            nc.vector.scalar_tensor_tensor(
                den16[:], cmb[:, 0:16], 1.0,
                denA[:].to_broadcast((128, 16)), ALU.mult, ALU.mult)
            t2m = plo.tile([128, 16], F32, tag="t2m")
            nc.vector.scalar_tensor_tensor(
                t2m[:], cmb[:, 16:32], 1.0,
                denB[:].to_broadcast((128, 16)), ALU.mult, ALU.mult)
            nc.vector.tensor_tensor(den16[:], den16[:], t2m[:], ALU.add)
            nc.vector.tensor_tensor(den16[:], den16[:], ps_add[:], ALU.add)

            civ = ccd_in[:].rearrange("a (b j) -> (a b) j", b=8)
            nc.gpsimd.dma_start(civ, den16[:])
            nc.gpsimd.collective_compute(
                "AllReduce", ALU.add, replica_groups=grp,
                ins=[ccd_in[:].opt()], outs=[ccd_out[:].opt()])

            # hide the natural_log table load under the AllReduce
            junk3 = pers.tile([128, 16], F32, tag="junk3")
            nc.scalar.activation(junk3[:], junk[:], AF.Ln)

            denf = plo.tile([128, 16], F32, tag="denf")
            cov = ccd_out[:].rearrange("a (b j) -> (a b) j", b=8)
            nc.gpsimd.dma_start(denf[:], cov)
            lnj = plo.tile([128, 16], F32, tag="lnj")
            lnacc = plo.tile([128, 1], F32, tag="lnacc")
            nc.scalar.activation(lnj[:], denf[:], AF.Ln, accum_out=lnacc[:])

            loss_ps = pps.tile([1, 1], F32, tag="ps", name="loss_ps")
            nc.tensor.matmul(loss_ps[:], lnacc[:], mselb[:],
                             start=True, stop=False)
            nc.tensor.matmul(loss_ps[:], possum[:], negf2[:],
                             start=False, stop=True)
            out_sb = pers.tile([1, 1], F32, tag="outsb")
            nc.vector.tensor_copy(out_sb[:], loss_ps[:])
            nc.sync.dma_start(y[:], out_sb[:])

    nc.compile()
    _CACHE["nc"] = nc
    return nc


def _make_inputs(emb_i, emb_j):
    e = np.concatenate([np.asarray(emb_i, np.float32),
                        np.asarray(emb_j, np.float32)], axis=0)
    sel = np.zeros((128, 2, 128), np.float32)
    for p in range(128):
        sel[p, :, np.arange(p % 16, 128, 16)] = 1.0
    sel = sel.reshape(128, 256).astype(ml_dtypes.float8_e4m3)
    eye = np.eye(128, dtype=np.float32).astype(ml_dtypes.bfloat16)

    in_maps = []
    for c in range(NCORES):
        loc = e[:, :, 16 * c:16 * (c + 1)]
        t = loc.reshape(R, 8, 8, 2, 16)
        t = t.transpose(1, 2, 4, 3, 0)
        x = np.ascontiguousarray(t).reshape(KT, 128, 2 * R).astype(
            ml_dtypes.float8_e4m3)

        wa2 = 1024 - 128 * c
        msk = np.zeros((2, 128, CH2), np.float32)
        msk[0, :, 0:wa2] = 1.0
        jd = 8 - c
        msk[1, np.arange(128), 128 * jd + np.arange(128)] = 1.0

        rt_m = np.zeros((16, 16), np.float32)
        for col in range(16):
            if col < 15 - c:
                rt_m[col, c + 1 + col] = 1.0
            elif col >= 16 - c:
                rt_m[col, col] = 1.0
        cm_m = np.zeros((2, 128, 16), np.float32)
        cm_m[0, :, c] = 1.0
        cm_m[1, :, 15 - c] = 1.0

        msel_m = np.zeros((128, 1), np.float32)
        msel_m[16 * c:16 * (c + 1), 0] = 1.0

        in_maps.append({
            "x": x, "sel": sel, "eye": eye,
            "msk": msk.astype(ml_dtypes.bfloat16),
            "rt": rt_m.astype(ml_dtypes.bfloat16),
            "cm": cm_m.astype(np.float32),
            "msel": msel_m,
        })
    return in_maps


def run(emb_i, emb_j, **spmd_kwargs):
    nc = _build_nc()
    in_maps = _make_inputs(emb_i, emb_j)
    res = bass_utils.run_bass_kernel_spmd(
        nc, in_maps, core_ids=list(range(NCORES)), **spmd_kwargs)
    total = sum(float(r["y"][0, 0]) for r in res.results)
    return np.array(total / R, dtype=np.float32), res


def kernel(emb_i, emb_j):
    loss, _ = run(emb_i, emb_j)
    return loss


# revision 25
# speedup vs baseline: 1.0227x; 1.0077x over previous
"""Contrastive loss on 8 TRN2 cores — v6 (fat-row A2A, lean front).

v5 -> v6: collective payloads redeclared [128, 8w] (byte-identical flat
layout, 8x fewer+fatter rows -> A2A drops to ~5us); x DMAs issued first
so tiles arrive at full HBM pace; h-split dropped (it caused ACT
table-set ping-pong: Square binds to exp_and_others, Abs_reciprocal_sqrt
to its own set); per-chunk post work interleaved into the ship loop so
exps ride the A2A chain instead of stacking at the tail.
"""

import numpy as np
import ml_dtypes

import concourse.bacc as bacc
import concourse.mybir as mybir
import concourse.tile as tile
from concourse import bass_utils

F32 = mybir.dt.float32
F8E5 = mybir.dt.float8e5
BF16 = mybir.dt.bfloat16
FP8 = mybir.dt.float8e4
AF = mybir.ActivationFunctionType
ALU = mybir.AluOpType
PM = mybir.MatmulPerfMode

B = 1024
R = 2 * B
NCORES = 8
KT = 8
S = 16.0
INV_T_S2 = 2.0 / (S * S)
CH1 = 1024
CH2 = 1152

CHUNKS = [("A", 0, 512), ("A", 512, 1024),
          ("B", 0, 512), ("B", 512, 1024), ("B", 1024, 1152)]

_CACHE = {}


def _pieces(c, lo, hi):
    """Segment pieces of cc2-local cols [lo,hi) for pair c."""
    wa2 = 1024 - 128 * c
    tb = 15 - c
    segs = []
    a0, a1 = max(lo, 0), min(hi, wa2)
    if a1 > a0:
        segs.append((a0 - lo, 128 * c + CH1 + a0, a1 - a0, c))
    b0, b1 = max(lo, wa2), min(hi, CH2)
    if b1 > b0:
        segs.append((b0 - lo, 128 * tb + (b0 - wa2), b1 - b0, tb))
    out = []
    for po, sc, w, tr in segs:
        off = 0
        while off < w:
            ww = min(512, w - off)
            out.append((po + off, sc + off, ww, tr))
            off += ww
    return out


def _chunk_units(space, lo, hi):
    units = []
    for u in range(8):
        if space == "A":
            units.append([(0, 128 * u + lo, hi - lo, u)])
        else:
            units.append(_pieces(u, lo, hi))
    return units


def _build_nc():
    if "nc" in _CACHE:
        return _CACHE["nc"]
    nc = bacc.Bacc("TRN2", target_bir_lowering=False, debug=False,
                   num_devices=NCORES)

    x = nc.dram_tensor("x", [KT, 128, 2 * R], FP8, kind="ExternalInput")
    sel = nc.dram_tensor("sel", [128, 256], FP8, kind="ExternalInput")
    eye = nc.dram_tensor("eye", [128, 128], BF16, kind="ExternalInput")
    msk = nc.dram_tensor("msk", [2, 128, CH2], BF16, kind="ExternalInput")
    rt = nc.dram_tensor("rt", [16, 16], BF16, kind="ExternalInput")
    cm = nc.dram_tensor("cm", [2, 128, 16], F32, kind="ExternalInput")
    msel = nc.dram_tensor("msel", [128, 1], F32, kind="ExternalInput")
    y = nc.dram_tensor("y", [1, 1], F32, kind="ExternalOutput")

    # fat-row collective payloads: [128, 8w]; shard u = rows [16u,16u+16)
    # holds the same flat bytes as rows [128u,128(u+1)) of a [1024, w]
    # layout, so unit-major views below stay byte-identical.
    cc = []
    for ci, (space, lo, hi) in enumerate(CHUNKS):
        w = hi - lo
        cin = nc.dram_tensor(f"cc{ci}_in", [128, 8 * w], F8E5)
        cout = nc.dram_tensor(f"cc{ci}_out", [128, 8 * w], F8E5)
        cc.append((cin, cout))
    ccd_in = nc.dram_tensor("ccd_in", [16, 128], F32)
    ccd_out = nc.dram_tensor("ccd_out", [16, 128], F32)
    wu_in = nc.dram_tensor("wu_in", [16, 16], F8E5)
    wu_out = nc.dram_tensor("wu_out", [16, 16], F8E5)
    grp = [list(range(NCORES))]

    with tile.TileContext(nc) as tc:
        with tc.tile_pool(name="x8", bufs=KT) as px8, \
             tc.tile_pool(name="sq", bufs=4) as psq, \
             tc.tile_pool(name="pers", bufs=1) as pers, \
             tc.tile_pool(name="simsb", bufs=2) as psim, \
             tc.tile_pool(name="acc", bufs=2) as pacc, \
             tc.tile_pool(name="sum", bufs=2) as psum_pool, \
             tc.tile_pool(name="loss", bufs=1) as plo, \
             tc.tile_pool(name="ps", bufs=7, space="PSUM") as pps:

            # ---- sel first (unblocks ssq), then x split across the two
            # HWDGE queues (SP + Activation) for 2x DMA service ----
            selb = pers.tile([128, 256], FP8, tag="selb")
            nc.scalar.dma_start(selb[:], sel[:])
            xb = []
            for k in range(KT):
                t = px8.tile([128, 2 * R], FP8, tag="x8")
                eng = nc.sync if k % 2 == 0 else nc.scalar
                eng.dma_start(t[:], x[k, :, :])
                xb.append(t)
            eyeb = pers.tile([128, 128], BF16, tag="eyeb")
            nc.sync.dma_start(eyeb[:], eye[:])
            mskb = pers.tile([128, 2 * CH2], BF16, tag="mskb")
            nc.sync.dma_start(mskb[:, 0:CH2], msk[0, :, :])
            nc.sync.dma_start(mskb[:, CH2:2 * CH2], msk[1, :, :])
            rtb = pers.tile([16, 16], BF16, tag="rtb")
            nc.sync.dma_start(rtb[:], rt[:])
            cmb = pers.tile([128, 32], F32, tag="cmb")
            nc.sync.dma_start(cmb[:, 0:16], cm[0, :, :])
            nc.sync.dma_start(cmb[:, 16:32], cm[1, :, :])
            mselb = pers.tile([128, 1], F32, tag="mselb")
            nc.sync.dma_start(mselb[:], msel[:])

            # ACT table warm: Square's set (exp_and_others)
            junk = pers.tile([128, 16], F32, tag="junk")
            nc.vector.memset(junk[:], 1.0)
            junko = pers.tile([128, 16], F32, tag="junko")
            nc.scalar.activation(junko[:], junk[:], AF.Square)

            ones1 = pers.tile([128, 1], BF16, tag="ones1")
            nc.vector.memset(ones1[:], 1.0)
            negf2 = pers.tile([128, 1], F32, tag="negf2")
            nc.vector.memset(negf2[:], -2.0 * INV_T_S2)

            wub = pers.tile([16, 16], F8E5, tag="wub")
            nc.vector.memset(wub[:], 1.0)
            nc.gpsimd.dma_start(wu_in[:], wub[:])
            nc.gpsimd.collective_compute(
                "AllToAll", ALU.bypass, replica_groups=grp,
                ins=[wu_in[:].opt()], outs=[wu_out[:].opt()])

            selv = selb[:].rearrange("p (two j) -> p two j", two=2)
            scale_t = pers.tile([128, R], FP8, tag="scale_t")

            def vk(k):
                return xb[k][:].rearrange("p (two r) -> p two r", two=2)

            # ---- squares (ACT even / DVE odd) + ssq matmuls ----
            ssq = [pps.tile([128, 512], F32, tag="ps", name=f"ssq{j}")
                   for j in range(4)]
            for k in range(KT):
                sq = psq.tile([128, 2 * R], FP8, tag="sq")
                src = xb[k][:]
                if k % 2 == 0:
                    nc.scalar.activation(sq[:], src, AF.Square)
                else:
                    nc.vector.tensor_tensor(sq[:], src, src, ALU.mult)
                sqv = sq[:].rearrange("p (two r) -> p two r", two=2)
                for j in range(4):
                    nc.tensor.matmul(ssq[j][:], selv,
                                     sqv[:, :, 512 * j:512 * (j + 1)],
                                     start=(k == 0), stop=(k == KT - 1),
                                     perf_mode=PM.DoubleRow)
            for j in range(4):
                nc.scalar.activation(scale_t[:, 512 * j:512 * (j + 1)],
                                     ssq[j][:], AF.Abs_reciprocal_sqrt,
                                     scale=128.0 / (S * S))
            # pull the exp/copy table set back in off the critical path
            nc.scalar.activation(junko[:], junk[:], AF.Exp)

            # gram machinery ------------------------------------------
            chunk_tiles = {}

            def gram_part(ci, uis, ks, unit_major=True):
                space, lo, hi = CHUNKS[ci]
                w = hi - lo
                units = _chunk_units(space, lo, hi)
                if ci not in chunk_tiles:
                    chunk_tiles[ci] = [None] * 8
                for ui in uis:
                    if chunk_tiles[ci][ui] is None:
                        chunk_tiles[ci][ui] = pps.tile(
                            [128, w], F32, tag="ps", name=f"g{ci}_{ui}")
                order = ([(ui, k) for ui in uis for k in ks] if unit_major
                         else [(ui, k) for k in ks for ui in uis])
                for ui, k in order:
                    v = vk(k)
                    pt = chunk_tiles[ci][ui]
                    for po, sc, ww, tr in units[ui]:
                        lhsT = v[:, :, 128 * tr:128 * (tr + 1)]
                        nc.tensor.matmul(pt[:, po:po + ww], lhsT,
                                         v[:, :, sc:sc + ww],
                                         start=(k == 0),
                                         stop=(k == KT - 1),
                                         perf_mode=PM.DoubleRow)

            def ship_chunk(ci):
                space, lo, hi = CHUNKS[ci]
                w = hi - lo
                cin, cout = cc[ci]
                wide = psim.tile([128, 8 * w], F8E5, tag="simsb",
                                 name=f"wide{ci}")
                for ui in range(8):
                    dst = wide[:, ui * w:(ui + 1) * w]
                    if ui % 2 == 0:
                        nc.scalar.activation(dst, chunk_tiles[ci][ui][:],
                                             AF.Copy)
                    else:
                        nc.vector.tensor_copy(dst, chunk_tiles[ci][ui][:])
                # unit-major [p, u, w] view of the fat-row [128, 8w] dram:
                # offset (u*128+p)*w, byte-identical to a [1024, w] layout
                cv = cin[:].rearrange("(u a2) (b w) -> (a2 b) u w",
                                      u=8, b=8)
                wv = wide[:].rearrange("p (u w) -> p u w", u=8)
                nc.gpsimd.dma_start(cv[:, 0:4, :], wv[:, 0:4, :])
                nc.gpsimd.dma_start(cv[:, 4:8, :], wv[:, 4:8, :])
                nc.gpsimd.collective_compute(
                    "AllToAll", ALU.bypass, replica_groups=grp,
                    ins=[cin[:].opt()], outs=[cout[:].opt()])

            # norm + chunk0 interleaved per k; then chunks 1-4
            for k in range(KT):
                for s in range(2):
                    sl = xb[k][:, s * R:(s + 1) * R]
                    nc.vector.tensor_tensor(sl, sl, scale_t[:], ALU.mult)
                gram_part(0, list(range(8)), [k], unit_major=False)
            ship_chunk(0)

            # ---- per-chunk post-collective loss work ----
            rs_parts = []
            rsA2_parts = []
            expdB_parts = []
            holders = {}
            pc_idx = 0
            pc_sb = plo.tile([128, 16], BF16, tag="pc_sb")

            def post_chunk(ci):
                nonlocal pc_idx
                space, lo, hi = CHUNKS[ci]
                w = hi - lo
                cin, cout = cc[ci]
                ld = psum_pool.tile([128, 8 * w], F8E5, tag="ld",
                                    name=f"ld{ci}", bufs=2)
                lv = ld[:].rearrange("p (s w) -> p s w", s=8)
                ov = cout[:].rearrange("(s a2) (b w) -> (a2 b) s w",
                                       s=8, b=8)
                nc.gpsimd.dma_start(lv[:, 0:4, :], ov[:, 0:4, :])
                nc.gpsimd.dma_start(lv[:, 4:8, :], ov[:, 4:8, :])
                t4 = []
                for a in range(4):
                    tt = psum_pool.tile([128, w], BF16, tag="t4",
                                        name=f"t4_{ci}_{a}", bufs=4)
                    nc.vector.tensor_tensor(tt[:], lv[:, 2 * a, :],
                                            lv[:, 2 * a + 1, :], ALU.add)
                    t4.append(tt)
                t2 = []
                for a in range(2):
                    tt = psum_pool.tile([128, w], BF16, tag="t2",
                                        name=f"t2_{ci}_{a}", bufs=2)
                    nc.vector.tensor_tensor(tt[:], t4[2 * a][:],
                                            t4[2 * a + 1][:], ALU.add)
                    t2.append(tt)
                sim = psum_pool.tile([128, w], BF16, tag="sim",
                                     name=f"sim{ci}", bufs=2)
                nc.vector.tensor_tensor(sim[:], t2[0][:], t2[1][:], ALU.add)

                if space == "B" and lo == 0:
                    scrP = pacc.tile([128, 128], BF16, tag="scrP", bufs=1)
                    possum = plo.tile([128, 1], F32, tag="possum")
                    nc.vector.scalar_tensor_tensor(
                        scrP[:], sim[:, 0:128], 1.0, eyeb[:],
                        ALU.mult, ALU.mult, accum_out=possum[:])
                    holders["possum"] = possum

                ex = psum_pool.tile([128, w], BF16, tag="ex",
                                    name=f"ex{ci}", bufs=5)
                rs = plo.tile([128, 1], F32, tag=f"rs{ci}")
                nc.scalar.activation(ex[:], sim[:], AF.Exp, scale=INV_T_S2,
                                     accum_out=rs[:])
                rs_parts.append((space, rs))

                if space == "A" and lo == 0:
                    scrA = pacc.tile([128, 128], BF16, tag="scrA", bufs=1)
                    expdA = plo.tile([128, 1], F32, tag="expdA")
                    nc.vector.scalar_tensor_tensor(
                        scrA[:], ex[:, 0:128], 1.0, eyeb[:],
                        ALU.mult, ALU.mult, accum_out=expdA[:])
                    holders["expdA"] = expdA
                if space == "B":
                    scr0 = pacc.tile([128, w], BF16, tag="scr0",
                                     name=f"scr0_{ci}", bufs=2)
                    ra = plo.tile([128, 1], F32, tag=f"ra{ci}")
                    nc.vector.scalar_tensor_tensor(
                        scr0[:], ex[:], 1.0, mskb[:, lo:hi],
                        ALU.mult, ALU.mult, accum_out=ra[:])
                    rsA2_parts.append(ra)
                    scr1 = pacc.tile([128, w], BF16, tag="scr1",
                                     name=f"scr1_{ci}", bufs=2)
                    rb = plo.tile([128, 1], F32, tag=f"rb{ci}")
                    nc.vector.scalar_tensor_tensor(
                        scr1[:], ex[:], 1.0, mskb[:, CH2 + lo:CH2 + hi],
                        ALU.mult, ALU.mult, accum_out=rb[:])
                    expdB_parts.append(rb)

                blocks = list(range(w // 128))
                if space == "A" and lo == 0:
                    blocks = blocks[1:]
                nb = len(blocks)
                ps4 = pps.tile([128, nb], F32, tag="pc", name=f"pc{ci}",
                               bufs=1)
                for bi, j in enumerate(blocks):
                    nc.tensor.matmul(ps4[:, bi:bi + 1],
                                     ex[:, 128 * j:128 * (j + 1)],
                                     ones1[:], start=True, stop=True)
                nc.scalar.activation(pc_sb[:, pc_idx:pc_idx + nb],
                                     ps4[:], AF.Copy)
                pc_idx += nb

            for ci in range(1, len(CHUNKS)):
                gram_part(ci, list(range(4)), list(range(KT)))
                gram_part(ci, list(range(4, 8)), list(range(KT)))
                ship_chunk(ci)
                post_chunk(ci - 1)
            post_chunk(len(CHUNKS) - 1)

            possum = holders["possum"]
            expdA = holders["expdA"]

            # colsum redistribution via one end transpose + rt matmul
            ps_t = pps.tile([128, 128], BF16, tag="ps", name="ps_t")
            nc.tensor.transpose(ps_t[0:16, :], pc_sb[:], eyeb[:])
            pt_sb = plo.tile([16, 128], BF16, tag="pt_sb")
            nc.vector.tensor_copy(pt_sb[:], ps_t[0:16, :])
            ps_add = pps.tile([128, 16], F32, tag="ps", name="ps_add")
            nc.tensor.matmul(ps_add[:], pt_sb[:], rtb[:], start=True,
                             stop=True)

            # ---- combine denominators ----
            denA = plo.tile([128, 1], F32, tag="denA")
            denB = plo.tile([128, 1], F32, tag="denB")
            rsA1 = plo.tile([128, 1], F32, tag="rsA1")
            a_parts = [r for sp, r in rs_parts if sp == "A"]
            b_parts = [r for sp, r in rs_parts if sp == "B"]
            nc.vector.tensor_tensor(rsA1[:], a_parts[0][:], a_parts[1][:],
                                    ALU.add)
            rs2t = plo.tile([128, 1], F32, tag="rs2t")
            nc.vector.tensor_tensor(rs2t[:], b_parts[0][:], b_parts[1][:],
                                    ALU.add)
            nc.vector.tensor_tensor(rs2t[:], rs2t[:], b_parts[2][:], ALU.add)
            rsA2 = plo.tile([128, 1], F32, tag="rsA2")
            nc.vector.tensor_tensor(rsA2[:], rsA2_parts[0][:],
                                    rsA2_parts[1][:], ALU.add)
            nc.vector.tensor_tensor(rsA2[:], rsA2[:], rsA2_parts[2][:],
                                    ALU.add)
            expdB = plo.tile([128, 1], F32, tag="expdB")
            nc.vector.tensor_tensor(expdB[:], expdB_parts[0][:],
                                    expdB_parts[1][:], ALU.add)
            nc.vector.tensor_tensor(expdB[:], expdB[:], expdB_parts[2][:],
                                    ALU.add)
            nc.vector.tensor_tensor(denA[:], rsA1[:], rsA2[:], ALU.add)
            nc.vector.tensor_sub(denA[:], denA[:], expdA[:])
            nc.vector.tensor_sub(denB[:], rs2t[:], rsA2[:])
            nc.vector.tensor_sub(denB[:], denB[:], expdB[:])

            den16 = plo.tile([128, 16], F32, tag="den16")
            nc.vector.scalar_tensor_tensor(
                den16[:], cmb[:, 0:16], 1.0,
                denA[:].to_broadcast((128, 16)), ALU.mult, ALU.mult)
            t2m = plo.tile([128, 16], F32, tag="t2m")
            nc.vector.scalar_tensor_tensor(
                t2m[:], cmb[:, 16:32], 1.0,
                denB[:].to_broadcast((128, 16)), ALU.mult, ALU.mult)
            nc.vector.tensor_tensor(den16[:], den16[:], t2m[:], ALU.add)
            nc.vector.tensor_tensor(den16[:], den16[:], ps_add[:], ALU.add)

            civ = ccd_in[:].rearrange("a (b j) -> (a b) j", b=8)
            nc.gpsimd.dma_start(civ, den16[:])
            nc.gpsimd.collective_compute(
                "AllReduce", ALU.add, replica_groups=grp,
                ins=[ccd_in[:].opt()], outs=[ccd_out[:].opt()])

            # hide the natural_log table load under the AllReduce
            junk3 = pers.tile([128, 16], F32, tag="junk3")
            nc.scalar.activation(junk3[:], junk[:], AF.Ln)

            denf = plo.tile([128, 16], F32, tag="denf")
            cov = ccd_out[:].rearrange("a (b j) -> (a b) j", b=8)
            nc.gpsimd.dma_start(denf[:], cov)
            lnj = plo.tile([128, 16], F32, tag="lnj")
            lnacc = plo.tile([128, 1], F32, tag="lnacc")
            nc.scalar.activation(lnj[:], denf[:], AF.Ln, accum_out=lnacc[:])

            loss_ps = pps.tile([1, 1], F32, tag="ps", name="loss_ps")
            nc.tensor.matmul(loss_ps[:], lnacc[:], mselb[:],
                             start=True, stop=False)
            nc.tensor.matmul(loss_ps[:], possum[:], negf2[:],
                             start=False, stop=True)
            out_sb = pers.tile([1, 1], F32, tag="outsb")
            nc.vector.tensor_copy(out_sb[:], loss_ps[:])
            nc.sync.dma_start(y[:], out_sb[:])

    nc.compile()
    _CACHE["nc"] = nc
    return nc


def _make_inputs(emb_i, emb_j):
    e = np.concatenate([np.asarray(emb_i, np.float32),
                        np.asarray(emb_j, np.float32)], axis=0)
    sel = np.zeros((128, 2, 128), np.float32)
    for p in range(128):
        sel[p, :, np.arange(p % 16, 128, 16)] = 1.0
    sel = sel.reshape(128, 256).astype(ml_dtypes.float8_e4m3)
    eye = np.eye(128, dtype=np.float32).astype(ml_dtypes.bfloat16)

    in_maps = []
    for c in range(NCORES):
        loc = e[:, :, 16 * c:16 * (c + 1)]
        t = loc.reshape(R, 8, 8, 2, 16)
        t = t.transpose(1, 2, 4, 3, 0)
        x = np.ascontiguousarray(t).reshape(KT, 128, 2 * R).astype(
            ml_dtypes.float8_e4m3)

        wa2 = 1024 - 128 * c
        msk = np.zeros((2, 128, CH2), np.float32)
        msk[0, :, 0:wa2] = 1.0
        jd = 8 - c
        msk[1, np.arange(128), 128 * jd + np.arange(128)] = 1.0

        rt_m = np.zeros((16, 16), np.float32)
        for col in range(16):
            if col < 15 - c:
                rt_m[col, c + 1 + col] = 1.0
            elif col >= 16 - c:
                rt_m[col, col] = 1.0
        cm_m = np.zeros((2, 128, 16), np.float32)
        cm_m[0, :, c] = 1.0
        cm_m[1, :, 15 - c] = 1.0

        msel_m = np.zeros((128, 1), np.float32)
        msel_m[16 * c:16 * (c + 1), 0] = 1.0

        in_maps.append({
            "x": x, "sel": sel, "eye": eye,
            "msk": msk.astype(ml_dtypes.bfloat16),
            "rt": rt_m.astype(ml_dtypes.bfloat16),
            "cm": cm_m.astype(np.float32),
            "msel": msel_m,
        })
    return in_maps


def run(emb_i, emb_j, **spmd_kwargs):
    nc = _build_nc()
    in_maps = _make_inputs(emb_i, emb_j)
    res = bass_utils.run_bass_kernel_spmd(
        nc, in_maps, core_ids=list(range(NCORES)), **spmd_kwargs)
    total = sum(float(r["y"][0, 0]) for r in res.results)
    return np.array(total / R, dtype=np.float32), res


def kernel(emb_i, emb_j):
    loss, _ = run(emb_i, emb_j)
    return loss
